# revision 1
# baseline (speedup 1.0000x reference)
"""Bass/Trainium2 kernel for nn_BayesianGNN (gnn_message_passing).

Computation (reference):
    agg1 = spmm(x, ew, src, dst)                       # [N, IN]
    for t in range(T):
        h_t   = relu(agg1 @ (W1 * mask1[t]))           # [N, HID]
        agg2  = spmm(h_t, ew, src, dst)                # [N, HID]
        out_t = agg2 @ (W2 * mask2[t])                 # [N, OUT]

Key algebraic restructure: spmm is linear, so
    out_t = spmm(relu(agg1 @ W1m_t) @ W2m_t)
i.e. the second spmm runs on OUT-wide (64) features instead of HID-wide
(128), and all T samples can be concatenated into one [N, T*OUT=512]
feature table P so the second spmm is done ONCE over 512-wide rows.

Sharding (8 cores): nodes are padded to 50176 = 392 blocks of 128 and
block-sharded: core c owns 49 consecutive blocks (6272 dst nodes) and all
edges pointing into them. Per-core work:

  KERNEL A: SpMM1 for its dst blocks via dma_gather of x rows + one-hot
    matmul segment-sum (S[e, j] = w_e * (dst_local_e == j), accumulated in
    PSUM per dst block, output transposed agg1T [feat, node]), then the
    dense per-sample MLP for its own nodes only, producing its shard of
    P [6272, 512] (node-major, samples in column groups of 64).
  (host) allgather of the P shards -> full P [50176, 512].
  KERNEL B: SpMM2: dma_gather of P rows by src + same one-hot matmul
    segment-sum -> out shard [6272, 512].
  (host) de-interleave -> [T, N, OUT].

dma_gather indices are int16, so gather tables are split in two halves of
25088 rows; per (block, half) the edge list is padded to a fixed number of
128-edge chunks (K_lo/K_hi, computed from the data) so the program is
identical across all 8 cores (SPMD), with padding slots gathering row 0
with weight 0.
"""

import sys

if "/opt/trn_rl_repo" not in sys.path:
    sys.path.insert(0, "/opt/trn_rl_repo")

import math

import numpy as np

import concourse.bass as bass
import concourse.tile as tile
from concourse import bacc, mybir
from concourse.bass import ts

F32 = mybir.dt.float32
BF16 = mybir.dt.bfloat16
I16 = mybir.dt.int16
I32 = mybir.dt.int32
USE_BF16 = True
DT = BF16 if USE_BF16 else F32

N, E = 50000, 800000
IN, HID, OUT, T = 96, 128, 64, 8
P = 128  # partitions
NCORES = 8
NBLK = 392  # node blocks of 128
NP_ = NBLK * P  # padded node count 50176
BPC = NBLK // NCORES  # blocks per core = 49
NPC = BPC * P  # nodes per core = 6272
HALF = NP_ // 2  # gather table half size = 25088
TO = T * OUT  # 512 = P feature width
NPRIME = 8  # leading blocks gathered at full count to flush stale SBUF slots


def _np_dt():
    if USE_BF16:
        import ml_dtypes

        return np.dtype(ml_dtypes.bfloat16)
    return np.dtype(np.float32)


def _pad_table(a, cols):
    """[n, c] -> zero-padded [NP_, cols], split into (lo, hi) halves."""
    out = np.zeros((NP_, cols), _np_dt())
    out[: a.shape[0], : a.shape[1]] = a
    return out[:HALF], out[HALF:]



# --------------------------------------------------------------------------
# host-side graph prep
# --------------------------------------------------------------------------
def prep_graph(src, dst, ew):
    """Partition + pad edges into per-(core, block, half) chunk schedules.

    Returns (K_lo, K_hi, per_core) where per_core[c] holds the int16 gather
    index arrays and the f32 dst-local / weight arrays in the layouts the
    kernels consume.
    """
    src = np.asarray(src).astype(np.int64).ravel()
    dst = np.asarray(dst).astype(np.int64).ravel()
    ew = np.asarray(ew, dtype=np.float32).ravel()

    blk = dst >> 7
    half = (src >= HALF).astype(np.int64)
    order = np.lexsort((src, half, blk))
    sblk = blk[order]
    shalf = half[order]
    ssrc = src[order]
    sew = ew[order]
    sdl = (dst[order] & 127).astype(np.float32)

    cell = sblk * 2 + shalf
    counts = np.bincount(cell, minlength=NBLK * 2)
    K_lo = int(math.ceil(counts[0::2].max() / P))
    K_hi = int(math.ceil(counts[1::2].max() / P))

    cell_starts = np.zeros(NBLK * 2 + 1, np.int64)
    np.cumsum(counts, out=cell_starts[1:])
    pos = np.arange(E, dtype=np.int64) - cell_starts[cell]

    b_local = sblk % BPC
    core = sblk // BPC

    per_core = []
    for c in range(NCORES):
        d = {}
        for s, K, tag in ((0, K_lo, "lo"), (1, K_hi, "hi")):
            m = (core == c) & (shalf == s)
            nslot = BPC * K * P
            # pad with -1 (skipped by the gather); the first NPRIME blocks
            # use full-count gathers to flush uninitialized buffer slots, so
            # their padding must be a valid row (0).
            idxf = np.full(nslot, -1, np.int16)
            df = np.zeros(nslot, np.float32)
            wf = np.zeros(nslot, np.float32)
            slots = b_local[m] * (K * P) + pos[m]
            idxf[slots] = (ssrc[m] - s * HALF).astype(np.int16)
            df[slots] = sdl[m]
            wf[slots] = sew[m]
            cnt = np.bincount(b_local[m], minlength=BPC).astype(np.int32)
            # never let a gather have zero valid indices
            empty = cnt == 0
            if empty.any():
                for b in np.nonzero(empty)[0]:
                    idxf[b * K * P] = 0
                cnt[empty] = 1
            d["idx_" + tag] = np.ascontiguousarray(
                np.tile(idxf.reshape(BPC * K * 8, 16).T, (8, 1))
            )
            d["cnt_" + tag] = np.ascontiguousarray(cnt.reshape(1, BPC))
            d["d_" + tag] = np.ascontiguousarray(
                df.reshape(BPC, K, P).transpose(2, 0, 1).reshape(P, BPC * K)
            )
            d["w_" + tag] = np.ascontiguousarray(
                wf.reshape(BPC, K, P).transpose(2, 0, 1).reshape(P, BPC * K)
            )
        per_core.append(d)
    cmin = {}
    for tag in ("lo", "hi"):
        allcnt = np.stack([pc["cnt_" + tag][0] for pc in per_core])  # [NCORES, BPC]
        cmin[tag] = (allcnt.min(axis=0) // P).astype(int).tolist()
    return K_lo, K_hi, per_core, cmin


def _iota_tile():
    return np.tile(np.arange(P, dtype=np.float32), (P, 1))


# --------------------------------------------------------------------------
# kernel A: SpMM1 (one-hot matmul segment-sum) + dense MLP -> P shard
# --------------------------------------------------------------------------
def build_kernel_a(K_lo, K_hi, cmin):
    ELEM = P
    nc = bacc.Bacc("TRN2", target_bir_lowering=False, debug=False)

    x_lo = nc.dram_tensor("x_lo", [HALF, P], DT, kind="ExternalInput")
    x_hi = nc.dram_tensor("x_hi", [HALF, P], DT, kind="ExternalInput")
    idx_lo = nc.dram_tensor("idx_lo", [P, BPC * K_lo * 8], I16, kind="ExternalInput")
    idx_hi = nc.dram_tensor("idx_hi", [P, BPC * K_hi * 8], I16, kind="ExternalInput")
    d_lo = nc.dram_tensor("d_lo", [P, BPC * K_lo], F32, kind="ExternalInput")
    w_lo = nc.dram_tensor("w_lo", [P, BPC * K_lo], F32, kind="ExternalInput")
    d_hi = nc.dram_tensor("d_hi", [P, BPC * K_hi], F32, kind="ExternalInput")
    w_hi = nc.dram_tensor("w_hi", [P, BPC * K_hi], F32, kind="ExternalInput")
    cnt_lo = nc.dram_tensor("cnt_lo", [1, BPC], I32, kind="ExternalInput")
    cnt_hi = nc.dram_tensor("cnt_hi", [1, BPC], I32, kind="ExternalInput")
    iota = nc.dram_tensor("iota", [P, P], F32, kind="ExternalInput")
    w1 = nc.dram_tensor("w1", [IN, HID], F32, kind="ExternalInput")
    m1 = nc.dram_tensor("m1", [IN, T * HID], F32, kind="ExternalInput")
    w2 = nc.dram_tensor("w2", [HID, OUT], F32, kind="ExternalInput")
    m2 = nc.dram_tensor("m2", [HID, T * OUT], F32, kind="ExternalInput")
    pshard = nc.dram_tensor("pshard", [NPC, TO], DT, kind="ExternalOutput")

    with tile.TileContext(nc) as tc:
        with (
            tc.tile_pool(name="const", bufs=1) as cpool,
            tc.tile_pool(name="glo", bufs=3) as gpool,
            tc.tile_pool(name="s", bufs=4) as spool,
            tc.tile_pool(name="h", bufs=3) as hpool,
            tc.tile_pool(name="po", bufs=2) as ppool,
            tc.tile_pool(name="acc", bufs=2, space="PSUM") as acc_pool,
            tc.tile_pool(name="ph", bufs=2, space="PSUM") as ph_pool,
            tc.tile_pool(name="pp", bufs=1, space="PSUM") as pp_pool,
        ):
            # ---- load constants
            def load(t_dram, shape, dtype=F32):
                nm = f"c_{t_dram.name}"
                t_sb = cpool.tile([P, shape[1]], dtype, name=nm, tag=nm)
                nc.sync.dma_start(out=t_sb[: shape[0], :], in_=t_dram[:])
                return t_sb

            idx_lo_t = load(idx_lo, [P, BPC * K_lo * 8], I16)
            idx_hi_t = load(idx_hi, [P, BPC * K_hi * 8], I16)
            d_lo_t = load(d_lo, [P, BPC * K_lo])
            w_lo_t = load(w_lo, [P, BPC * K_lo])
            d_hi_t = load(d_hi, [P, BPC * K_hi])
            w_hi_t = load(w_hi, [P, BPC * K_hi])
            cnt_lo_t = load(cnt_lo, [1, BPC], I32)
            cnt_hi_t = load(cnt_hi, [1, BPC], I32)
            iota_t = load(iota, [P, P])
            ztail = max(
                [(K_lo - c) for c in cmin["lo"]] + [(K_hi - c) for c in cmin["hi"]]
            )
            zero_t = cpool.tile([P, ztail * ELEM], DT, name="zero_t", tag="zero_t")
            nc.vector.memset(zero_t[:], 0.0)
            creg = {
                "lo": nc.gpsimd.alloc_register("cnt_reg_lo"),
                "hi": nc.gpsimd.alloc_register("cnt_reg_hi"),
            }
            cnt_tiles = {"lo": cnt_lo_t, "hi": cnt_hi_t}
            w1_t = load(w1, [IN, HID])
            m1_t = load(m1, [IN, T * HID])
            w2_t = load(w2, [HID, OUT])
            m2_t = load(m2, [HID, T * OUT])

            # masked weights; rows IN..P of w1m stay zero
            w1m = cpool.tile([P, T * HID], DT)
            nc.gpsimd.memset(w1m[:], 0.0)
            for t in range(T):
                nc.vector.tensor_tensor(
                    out=w1m[:IN, ts(t, HID)],
                    in0=w1_t[:IN, :],
                    in1=m1_t[:IN, ts(t, HID)],
                    op=mybir.AluOpType.mult,
                )
            w2m = cpool.tile([P, T * OUT], DT)
            for t in range(T):
                nc.vector.tensor_tensor(
                    out=w2m[:, ts(t, OUT)],
                    in0=w2_t[:, :],
                    in1=m2_t[:, ts(t, OUT)],
                    op=mybir.AluOpType.mult,
                )

            # agg1 transposed [feat, node] for the whole shard, kept in SBUF
            agg1t = cpool.tile([P, NPC], DT)

            halves = (
                (K_lo, x_lo, idx_lo_t, d_lo_t, w_lo_t, "glo", "lo"),
                (K_hi, x_hi, idx_hi_t, d_hi_t, w_hi_t, "ghi", "hi"),
            )

            def spmm1_block(b):
                acc = acc_pool.tile([P, P], F32, space="PSUM", tag="acc", name="acc")
                nmm = K_lo + K_hi
                i_mm = 0
                for K, xtab, idx_t, d_t, w_t, gtag, hkey in halves:
                    g = gpool.tile([P, K * ELEM], DT, tag=gtag, name=gtag)
                    cm = cmin[hkey][b]
                    if cm < K:
                        nc.scalar.activation(
                            out=g[:, cm * ELEM : K * ELEM],
                            in_=zero_t[:, : (K - cm) * ELEM],
                            func=mybir.ActivationFunctionType.Relu,
                        )
                    nreg = creg[hkey]
                    nc.gpsimd.reg_load(nreg, cnt_tiles[hkey][0:1, b : b + 1])
                    nc.gpsimd.dma_gather(
                        g[:].rearrange("p (k e) -> p k e", e=ELEM),
                        xtab[:],
                        idx_t[:, b * K * 8 : (b + 1) * K * 8],
                        K * P,
                        nreg,
                        ELEM,
                        single_packet=False,
                    )
                    for k in range(K):
                        s_t = spool.tile([P, P], DT, tag="s", name="s_t")
                        nc.vector.tensor_scalar(
                            out=s_t[:],
                            in0=iota_t[:],
                            scalar1=d_t[:, b * K + k : b * K + k + 1],
                            scalar2=w_t[:, b * K + k : b * K + k + 1],
                            op0=mybir.AluOpType.is_equal,
                            op1=mybir.AluOpType.mult,
                        )
                        # agg1t_block[feat, dstlocal] += g_chunk.T @ S
                        nc.tensor.matmul(
                            out=acc[:],
                            lhsT=g[:, ts(k, ELEM)],
                            rhs=s_t[:],
                            start=(i_mm == 0),
                            stop=(i_mm == nmm - 1),
                        )
                        i_mm += 1
                nc.vector.tensor_copy(out=agg1t[:, ts(b, P)], in_=acc[:])

            def dense_tile(off, w_):
                nj = w_ // P
                psum_p = [
                    pp_pool.tile([P, TO], F32, space="PSUM", tag=f"pp{j}", name=f"pp{j}")
                    for j in range(nj)
                ]
                for t in range(T):
                    psum_h = ph_pool.tile(
                        [P, w_], F32, space="PSUM", tag="ph", name="ph"
                    )
                    nc.tensor.matmul(
                        out=psum_h[:],
                        lhsT=w1m[:, ts(t, HID)],
                        rhs=agg1t[:, off : off + w_],
                        start=True,
                        stop=True,
                    )
                    h_sb = hpool.tile([P, w_], DT, tag="h", name="h_sb")
                    nc.scalar.activation(
                        out=h_sb[:],
                        in_=psum_h[:],
                        func=mybir.ActivationFunctionType.Relu,
                    )
                    for j in range(nj):
                        nc.tensor.matmul(
                            out=psum_p[j][:, ts(t, OUT)],
                            lhsT=h_sb[:, ts(j, P)],
                            rhs=w2m[:, ts(t, OUT)],
                            start=True,
                            stop=True,
                        )
                for j in range(nj):
                    p_sb = ppool.tile([P, TO], DT, tag="po", name="p_sb")
                    nc.scalar.copy(out=p_sb[:], in_=psum_p[j][:])
                    nc.sync.dma_start(
                        out=pshard[off + j * P : off + (j + 1) * P, :], in_=p_sb[:]
                    )

            # interleave: emit each dense 512-node tile right after the 4
            # spmm blocks that produce its agg1t columns
            st_widths = []
            off = 0
            while off < NPC:
                w_ = min(512, NPC - off)
                st_widths.append((off, w_))
                off += w_
            b = 0
            for off, w_ in st_widths:
                while b * P < off + w_:
                    spmm1_block(b)
                    b += 1
                dense_tile(off, w_)

    nc.compile()
    return nc


# --------------------------------------------------------------------------
# kernel B: SpMM2 over P -> out shard
# --------------------------------------------------------------------------
def build_kernel_b(K_lo, K_hi, cmin):
    ELEM = TO
    nc = bacc.Bacc("TRN2", target_bir_lowering=False, debug=False)

    p_lo = nc.dram_tensor("p_lo", [HALF, TO], DT, kind="ExternalInput")
    p_hi = nc.dram_tensor("p_hi", [HALF, TO], DT, kind="ExternalInput")
    idx_lo = nc.dram_tensor("idx_lo", [P, BPC * K_lo * 8], I16, kind="ExternalInput")
    idx_hi = nc.dram_tensor("idx_hi", [P, BPC * K_hi * 8], I16, kind="ExternalInput")
    d_lo = nc.dram_tensor("d_lo", [P, BPC * K_lo], F32, kind="ExternalInput")
    w_lo = nc.dram_tensor("w_lo", [P, BPC * K_lo], F32, kind="ExternalInput")
    d_hi = nc.dram_tensor("d_hi", [P, BPC * K_hi], F32, kind="ExternalInput")
    w_hi = nc.dram_tensor("w_hi", [P, BPC * K_hi], F32, kind="ExternalInput")
    cnt_lo = nc.dram_tensor("cnt_lo", [1, BPC], I32, kind="ExternalInput")
    cnt_hi = nc.dram_tensor("cnt_hi", [1, BPC], I32, kind="ExternalInput")
    iota = nc.dram_tensor("iota", [P, P], F32, kind="ExternalInput")
    oshard = nc.dram_tensor("oshard", [NPC, TO], F32, kind="ExternalOutput")

    with tile.TileContext(nc) as tc:
        with (
            tc.tile_pool(name="const", bufs=1) as cpool,
            tc.tile_pool(name="g", bufs=3) as gpool,
            tc.tile_pool(name="s", bufs=4) as spool,
            tc.tile_pool(name="o", bufs=3) as opool,
            tc.tile_pool(name="po", bufs=6, space="PSUM") as po_pool,
        ):
            def load(t_dram, shape, dtype=F32):
                nm = f"c_{t_dram.name}"
                t_sb = cpool.tile([P, shape[1]], dtype, name=nm, tag=nm)
                nc.sync.dma_start(out=t_sb[: shape[0], :], in_=t_dram[:])
                return t_sb

            idx_lo_t = load(idx_lo, [P, BPC * K_lo * 8], I16)
            idx_hi_t = load(idx_hi, [P, BPC * K_hi * 8], I16)
            d_lo_t = load(d_lo, [P, BPC * K_lo])
            w_lo_t = load(w_lo, [P, BPC * K_lo])
            d_hi_t = load(d_hi, [P, BPC * K_hi])
            w_hi_t = load(w_hi, [P, BPC * K_hi])
            cnt_lo_t = load(cnt_lo, [1, BPC], I32)
            cnt_hi_t = load(cnt_hi, [1, BPC], I32)
            iota_t = load(iota, [P, P])
            ztail = max(
                [(K_lo - c) for c in cmin["lo"]] + [(K_hi - c) for c in cmin["hi"]]
            )
            zero_t = cpool.tile([P, ztail * ELEM], DT, name="zero_t", tag="zero_t")
            nc.vector.memset(zero_t[:], 0.0)
            creg = {
                "lo": nc.gpsimd.alloc_register("cnt_reg_lo"),
                "hi": nc.gpsimd.alloc_register("cnt_reg_hi"),
            }
            cnt_tiles = {"lo": cnt_lo_t, "hi": cnt_hi_t}

            halves = (
                (K_lo, p_lo, idx_lo_t, d_lo_t, w_lo_t, "glo", "lo"),
                (K_hi, p_hi, idx_hi_t, d_hi_t, w_hi_t, "ghi", "hi"),
            )
            for b in range(BPC):
                acc = po_pool.tile([P, TO], F32, space="PSUM", tag="acc")
                nmm = K_lo + K_hi
                i_mm = 0
                for K, ptab, idx_t, d_t, w_t, gtag, hkey in halves:
                    g = gpool.tile([P, K * ELEM], DT, tag=gtag)
                    cm = cmin[hkey][b]
                    if cm < K:
                        nc.scalar.activation(
                            out=g[:, cm * ELEM : K * ELEM],
                            in_=zero_t[:, : (K - cm) * ELEM],
                            func=mybir.ActivationFunctionType.Relu,
                        )
                    nreg = creg[hkey]
                    nc.gpsimd.reg_load(nreg, cnt_tiles[hkey][0:1, b : b + 1])
                    nc.gpsimd.dma_gather(
                        g[:].rearrange("p (k e) -> p k e", e=ELEM),
                        ptab[:],
                        idx_t[:, b * K * 8 : (b + 1) * K * 8],
                        K * P,
                        nreg,
                        ELEM,
                        single_packet=False,
                    )
                    for k in range(K):
                        s_t = spool.tile([P, P], DT, tag="s")
                        nc.vector.tensor_scalar(
                            out=s_t[:],
                            in0=iota_t[:],
                            scalar1=d_t[:, b * K + k : b * K + k + 1],
                            scalar2=w_t[:, b * K + k : b * K + k + 1],
                            op0=mybir.AluOpType.is_equal,
                            op1=mybir.AluOpType.mult,
                        )
                        # out_block[dstlocal, f] += S.T @ g_chunk
                        nc.tensor.matmul(
                            out=acc[:],
                            lhsT=s_t[:],
                            rhs=g[:, ts(k, ELEM)],
                            start=(i_mm == 0),
                            stop=(i_mm == nmm - 1),
                        )
                        i_mm += 1
                o_sb = opool.tile([P, TO], F32, tag="o")
                nc.vector.tensor_copy(out=o_sb[:], in_=acc[:])
                nc.sync.dma_start(out=oshard[ts(b, P), :], in_=o_sb[:])

    nc.compile()
    return nc


# --------------------------------------------------------------------------
# host orchestration
# --------------------------------------------------------------------------
def prep_inputs(x, edge_weight, W1, W2, mask1, mask2, src, dst):
    K_lo, K_hi, per_core, cmin = prep_graph(src, dst, edge_weight)
    x_lo, x_hi = _pad_table(np.asarray(x, np.float32), P)
    iota = _iota_tile()
    w1 = np.ascontiguousarray(np.asarray(W1, np.float32))
    w2 = np.ascontiguousarray(np.asarray(W2, np.float32))
    m1 = np.ascontiguousarray(
        np.asarray(mask1, np.float32).transpose(1, 0, 2).reshape(IN, T * HID)
    )
    m2 = np.ascontiguousarray(
        np.asarray(mask2, np.float32).transpose(1, 0, 2).reshape(HID, T * OUT)
    )

    in_maps_a = []
    for c in range(NCORES):
        m = dict(per_core[c])
        m.update(
            x_lo=x_lo, x_hi=x_hi, iota=iota, w1=w1, m1=m1, w2=w2, m2=m2
        )
        in_maps_a.append(m)
    return K_lo, K_hi, cmin, per_core, in_maps_a


def assemble_output(oshards):
    full = np.concatenate(oshards, axis=0)  # [NP_, 512]
    return np.ascontiguousarray(
        full[:N].reshape(N, T, OUT).transpose(1, 0, 2)
    ).astype(np.float32)


def kernel(x, edge_weight, W1, W2, mask1, mask2, src, dst):
    from concourse.bass_utils import run_bass_kernel_spmd

    K_lo, K_hi, cmin, per_core, in_maps_a = prep_inputs(
        x, edge_weight, W1, W2, mask1, mask2, src, dst
    )

    nc_a = build_kernel_a(K_lo, K_hi, cmin)
    res_a = run_bass_kernel_spmd(nc_a, in_maps_a, core_ids=list(range(NCORES)))
    pshards = [res_a.results[c]["pshard"] for c in range(NCORES)]

    p_full = np.concatenate(pshards, axis=0)  # [NP_, 512]
    p_lo = np.ascontiguousarray(p_full[:HALF])
    p_hi = np.ascontiguousarray(p_full[HALF:])

    nc_b = build_kernel_b(K_lo, K_hi, cmin)
    in_maps_b = []
    for c in range(NCORES):
        m = dict(per_core[c])
        m.update(p_lo=p_lo, p_hi=p_hi, iota=_iota_tile())
        in_maps_b.append(m)
    res_b = run_bass_kernel_spmd(nc_b, in_maps_b, core_ids=list(range(NCORES)))
    oshards = [res_b.results[c]["oshard"] for c in range(NCORES)]

    return assemble_output(oshards)



# revision 2
# speedup vs baseline: 1.9057x; 1.9057x over previous
"""Bass/Trainium2 kernel for nn_BayesianGNN (gnn_message_passing).

Computation (reference):
    agg1 = spmm(x, ew, src, dst)                       # [N, IN]
    for t in range(T):
        h_t   = relu(agg1 @ (W1 * mask1[t]))           # [N, HID]
        agg2  = spmm(h_t, ew, src, dst)                # [N, HID]
        out_t = agg2 @ (W2 * mask2[t])                 # [N, OUT]

Key algebraic restructure: spmm is linear, so
    out_t = spmm(relu(agg1 @ W1m_t) @ W2m_t)
i.e. the second spmm runs on OUT-wide (64) features instead of HID-wide
(128), and all T samples can be concatenated into one [N, T*OUT=512]
feature table P so the second spmm is done ONCE over 512-wide rows.

Sharding (8 cores): nodes are padded to 50176 = 392 blocks of 128 and
block-sharded: core c owns 49 consecutive blocks (6272 dst nodes) and all
edges pointing into them. ONE fused kernel launch per run (the axon/PJRT
dispatch has a large fixed latency, so launch count dominates wall time):

  phase 1: SpMM1 for its dst blocks via dma_gather of x rows + one-hot
    matmul segment-sum (S[e, j] = w_e * (dst_local_e == j), accumulated in
    PSUM per dst block, output transposed agg1T [feat, node]), then the
    dense per-sample MLP for its own nodes only, producing its shard of
    P [6272, 512] (node-major, samples in column groups of 64) into a
    DRAM bounce buffer.
  AllGather (on-device collective) of the P shards -> full P [50176, 512]
    in DRAM on every core.
  phase 2: SpMM2: dma_gather of P rows by src + the same one-hot matmul
    segment-sum -> out shard [6272, 512] (ExternalOutput).
  (host) de-interleave -> [T, N, OUT].

dma_gather indices are int16, so gather tables are split in two halves of
25088 rows; per (block, half) the edge list is padded to a fixed number of
128-edge chunks (K_lo/K_hi, computed from the data) so the program is
identical across all 8 cores (SPMD), with padding slots skipped via a
count register and the stale tail zero-filled up to the cross-core
minimum count.
"""

import sys

if "/opt/trn_rl_repo" not in sys.path:
    sys.path.insert(0, "/opt/trn_rl_repo")

import math

import numpy as np

import concourse.bass as bass
import concourse.tile as tile
from concourse import bacc, mybir
from concourse.bass import ts

F32 = mybir.dt.float32
BF16 = mybir.dt.bfloat16
I16 = mybir.dt.int16
I32 = mybir.dt.int32
USE_BF16 = True
DT = BF16 if USE_BF16 else F32

N, E = 50000, 800000
IN, HID, OUT, T = 96, 128, 64, 8
P = 128  # partitions
NCORES = 8
NBLK = 392  # node blocks of 128
NP_ = NBLK * P  # padded node count 50176
BPC = NBLK // NCORES  # blocks per core = 49
NPC = BPC * P  # nodes per core = 6272
HALF = NP_ // 2  # gather table half size = 25088
TO = T * OUT  # 512 = P feature width


def _np_dt():
    if USE_BF16:
        import ml_dtypes

        return np.dtype(ml_dtypes.bfloat16)
    return np.dtype(np.float32)


def _pad_table(a, cols):
    """[n, c] -> zero-padded [NP_, cols], split into (lo, hi) halves."""
    out = np.zeros((NP_, cols), _np_dt())
    out[: a.shape[0], : a.shape[1]] = a
    return out[:HALF], out[HALF:]


# --------------------------------------------------------------------------
# host-side graph prep
# --------------------------------------------------------------------------
def prep_graph(src, dst, ew):
    """Partition + pad edges into per-(core, block, half) chunk schedules.

    Returns (K_lo, K_hi, per_core, cmin) where per_core[c] holds the int16
    gather index arrays and the f32 dst-local / weight arrays in the
    layouts the kernel consumes.
    """
    src = np.asarray(src).astype(np.int64).ravel()
    dst = np.asarray(dst).astype(np.int64).ravel()
    ew = np.asarray(ew, dtype=np.float32).ravel()

    blk = dst >> 7
    half = (src >= HALF).astype(np.int64)
    order = np.lexsort((src, half, blk))
    sblk = blk[order]
    shalf = half[order]
    ssrc = src[order]
    sew = ew[order]
    sdl = (dst[order] & 127).astype(np.float32)

    cell = sblk * 2 + shalf
    counts = np.bincount(cell, minlength=NBLK * 2)
    K_lo = int(math.ceil(counts[0::2].max() / P))
    K_hi = int(math.ceil(counts[1::2].max() / P))

    cell_starts = np.zeros(NBLK * 2 + 1, np.int64)
    np.cumsum(counts, out=cell_starts[1:])
    pos = np.arange(E, dtype=np.int64) - cell_starts[cell]

    b_local = sblk % BPC
    core = sblk // BPC

    per_core = []
    for c in range(NCORES):
        d = {}
        for s, K, tag in ((0, K_lo, "lo"), (1, K_hi, "hi")):
            m = (core == c) & (shalf == s)
            nslot = BPC * K * P
            # pad with -1 (skipped by the gather)
            idxf = np.full(nslot, -1, np.int16)
            df = np.zeros(nslot, np.float32)
            wf = np.zeros(nslot, np.float32)
            slots = b_local[m] * (K * P) + pos[m]
            idxf[slots] = (ssrc[m] - s * HALF).astype(np.int16)
            df[slots] = sdl[m]
            wf[slots] = sew[m]
            cnt = np.bincount(b_local[m], minlength=BPC).astype(np.int32)
            # never let a gather have zero valid indices
            empty = cnt == 0
            if empty.any():
                for b in np.nonzero(empty)[0]:
                    idxf[b * K * P] = 0
                cnt[empty] = 1
            d["idx_" + tag] = np.ascontiguousarray(
                np.tile(idxf.reshape(BPC * K * 8, 16).T, (8, 1))
            )
            d["cnt_" + tag] = np.ascontiguousarray(cnt.reshape(1, BPC))
            d["d_" + tag] = np.ascontiguousarray(
                df.reshape(BPC, K, P).transpose(2, 0, 1).reshape(P, BPC * K)
            )
            d["w_" + tag] = np.ascontiguousarray(
                wf.reshape(BPC, K, P).transpose(2, 0, 1).reshape(P, BPC * K)
            )
        per_core.append(d)
    cmin = {}
    for tag in ("lo", "hi"):
        allcnt = np.stack([pc["cnt_" + tag][0] for pc in per_core])  # [NCORES, BPC]
        cmin[tag] = (allcnt.min(axis=0) // P).astype(int).tolist()
    return K_lo, K_hi, per_core, cmin


def _iota_tile():
    return np.tile(np.arange(P, dtype=np.float32), (P, 1))


# --------------------------------------------------------------------------
# fused kernel: SpMM1 + MLP -> P shard; AllGather; SpMM2 -> out shard
# --------------------------------------------------------------------------
def build_kernel_fused(K_lo, K_hi, cmin, ncores=NCORES):
    nc = bacc.Bacc("TRN2", target_bir_lowering=False, debug=False)

    x_lo = nc.dram_tensor("x_lo", [HALF, P], DT, kind="ExternalInput")
    x_hi = nc.dram_tensor("x_hi", [HALF, P], DT, kind="ExternalInput")
    idx_lo = nc.dram_tensor("idx_lo", [P, BPC * K_lo * 8], I16, kind="ExternalInput")
    idx_hi = nc.dram_tensor("idx_hi", [P, BPC * K_hi * 8], I16, kind="ExternalInput")
    d_lo = nc.dram_tensor("d_lo", [P, BPC * K_lo], F32, kind="ExternalInput")
    w_lo = nc.dram_tensor("w_lo", [P, BPC * K_lo], F32, kind="ExternalInput")
    d_hi = nc.dram_tensor("d_hi", [P, BPC * K_hi], F32, kind="ExternalInput")
    w_hi = nc.dram_tensor("w_hi", [P, BPC * K_hi], F32, kind="ExternalInput")
    cnt_lo = nc.dram_tensor("cnt_lo", [1, BPC], I32, kind="ExternalInput")
    cnt_hi = nc.dram_tensor("cnt_hi", [1, BPC], I32, kind="ExternalInput")
    iota = nc.dram_tensor("iota", [P, P], F32, kind="ExternalInput")
    w1 = nc.dram_tensor("w1", [IN, HID], F32, kind="ExternalInput")
    m1 = nc.dram_tensor("m1", [IN, T * HID], F32, kind="ExternalInput")
    w2 = nc.dram_tensor("w2", [HID, OUT], F32, kind="ExternalInput")
    m2 = nc.dram_tensor("m2", [HID, T * OUT], F32, kind="ExternalInput")
    oshard = nc.dram_tensor("oshard", [NPC, TO], F32, kind="ExternalOutput")

    with tile.TileContext(nc) as tc:
        with (
            tc.tile_pool(name="const", bufs=1) as cpool,
            tc.tile_pool(name="dram", bufs=1, space="DRAM") as dpool,
            tc.tile_pool(name="s", bufs=4) as spool,
        ):
            pshard_d = dpool.tile([NPC, TO], DT, name="pshard_d")
            pfull_d = dpool.tile([NP_, TO], DT, name="pfull_d")

            # ---- load constants
            def load(t_dram, shape, dtype=F32):
                nm = f"c_{t_dram.name}"
                t_sb = cpool.tile([P, shape[1]], dtype, name=nm, tag=nm)
                nc.sync.dma_start(out=t_sb[: shape[0], :], in_=t_dram[:])
                return t_sb

            idx_lo_t = load(idx_lo, [P, BPC * K_lo * 8], I16)
            idx_hi_t = load(idx_hi, [P, BPC * K_hi * 8], I16)
            d_lo_t = load(d_lo, [P, BPC * K_lo])
            w_lo_t = load(w_lo, [P, BPC * K_lo])
            d_hi_t = load(d_hi, [P, BPC * K_hi])
            w_hi_t = load(w_hi, [P, BPC * K_hi])
            cnt_lo_t = load(cnt_lo, [1, BPC], I32)
            cnt_hi_t = load(cnt_hi, [1, BPC], I32)
            iota_t = load(iota, [P, P])
            ztail = max(
                [(K_lo - c) for c in cmin["lo"]] + [(K_hi - c) for c in cmin["hi"]]
            )
            # zero tail sized for the widest use (phase 2, ELEM=TO)
            zero_t = cpool.tile([P, ztail * TO], DT, name="zero_t", tag="zero_t")
            nc.vector.memset(zero_t[:], 0.0)
            creg = {
                "lo": nc.gpsimd.alloc_register("cnt_reg_lo"),
                "hi": nc.gpsimd.alloc_register("cnt_reg_hi"),
            }
            cnt_tiles = {"lo": cnt_lo_t, "hi": cnt_hi_t}
            w1_t = load(w1, [IN, HID])
            m1_t = load(m1, [IN, T * HID])
            w2_t = load(w2, [HID, OUT])
            m2_t = load(m2, [HID, T * OUT])

            # masked weights; rows IN..P of w1m stay zero
            w1m = cpool.tile([P, T * HID], DT)
            nc.gpsimd.memset(w1m[:], 0.0)
            for t in range(T):
                nc.vector.tensor_tensor(
                    out=w1m[:IN, ts(t, HID)],
                    in0=w1_t[:IN, :],
                    in1=m1_t[:IN, ts(t, HID)],
                    op=mybir.AluOpType.mult,
                )
            w2m = cpool.tile([P, T * OUT], DT)
            for t in range(T):
                nc.vector.tensor_tensor(
                    out=w2m[:, ts(t, OUT)],
                    in0=w2_t[:, :],
                    in1=m2_t[:, ts(t, OUT)],
                    op=mybir.AluOpType.mult,
                )

            # agg1 transposed [feat, node] for the whole shard, kept in SBUF
            agg1t = cpool.tile([P, NPC], DT)

            # ---------------- phase 1: SpMM1 + dense MLP -> pshard_d ----
            with (
                tc.tile_pool(name="glo1", bufs=3) as gpool,
                tc.tile_pool(name="h", bufs=3) as hpool,
                tc.tile_pool(name="po", bufs=2) as ppool,
                tc.tile_pool(name="acc", bufs=2, space="PSUM") as acc_pool,
                tc.tile_pool(name="ph", bufs=2, space="PSUM") as ph_pool,
                tc.tile_pool(name="pp", bufs=1, space="PSUM") as pp_pool,
            ):
                halves1 = (
                    (K_lo, x_lo, idx_lo_t, d_lo_t, w_lo_t, "glo", "lo"),
                    (K_hi, x_hi, idx_hi_t, d_hi_t, w_hi_t, "ghi", "hi"),
                )

                def spmm1_block(b):
                    ELEM = P
                    acc = acc_pool.tile([P, P], F32, space="PSUM", tag="acc", name="acc")
                    nmm = K_lo + K_hi
                    i_mm = 0
                    for K, xtab, idx_t, d_t, w_t, gtag, hkey in halves1:
                        g = gpool.tile([P, K * ELEM], DT, tag=gtag, name=gtag)
                        cm = cmin[hkey][b]
                        if cm < K:
                            nc.scalar.activation(
                                out=g[:, cm * ELEM : K * ELEM],
                                in_=zero_t[:, : (K - cm) * ELEM],
                                func=mybir.ActivationFunctionType.Relu,
                            )
                        nreg = creg[hkey]
                        nc.gpsimd.reg_load(nreg, cnt_tiles[hkey][0:1, b : b + 1])
                        nc.gpsimd.dma_gather(
                            g[:].rearrange("p (k e) -> p k e", e=ELEM),
                            xtab[:],
                            idx_t[:, b * K * 8 : (b + 1) * K * 8],
                            K * P,
                            nreg,
                            ELEM,
                            single_packet=False,
                        )
                        for k in range(K):
                            s_t = spool.tile([P, P], DT, tag="s", name="s_t")
                            nc.vector.tensor_scalar(
                                out=s_t[:],
                                in0=iota_t[:],
                                scalar1=d_t[:, b * K + k : b * K + k + 1],
                                scalar2=w_t[:, b * K + k : b * K + k + 1],
                                op0=mybir.AluOpType.is_equal,
                                op1=mybir.AluOpType.mult,
                            )
                            # agg1t_block[feat, dstlocal] += g_chunk.T @ S
                            nc.tensor.matmul(
                                out=acc[:],
                                lhsT=g[:, ts(k, ELEM)],
                                rhs=s_t[:],
                                start=(i_mm == 0),
                                stop=(i_mm == nmm - 1),
                            )
                            i_mm += 1
                    nc.vector.tensor_copy(out=agg1t[:, ts(b, P)], in_=acc[:])

                def dense_tile(off, w_):
                    nj = w_ // P
                    psum_p = [
                        pp_pool.tile(
                            [P, TO], F32, space="PSUM", tag=f"pp{j}", name=f"pp{j}"
                        )
                        for j in range(nj)
                    ]
                    for t in range(T):
                        psum_h = ph_pool.tile(
                            [P, w_], F32, space="PSUM", tag="ph", name="ph"
                        )
                        nc.tensor.matmul(
                            out=psum_h[:],
                            lhsT=w1m[:, ts(t, HID)],
                            rhs=agg1t[:, off : off + w_],
                            start=True,
                            stop=True,
                        )
                        h_sb = hpool.tile([P, w_], DT, tag="h", name="h_sb")
                        nc.scalar.activation(
                            out=h_sb[:],
                            in_=psum_h[:],
                            func=mybir.ActivationFunctionType.Relu,
                        )
                        for j in range(nj):
                            nc.tensor.matmul(
                                out=psum_p[j][:, ts(t, OUT)],
                                lhsT=h_sb[:, ts(j, P)],
                                rhs=w2m[:, ts(t, OUT)],
                                start=True,
                                stop=True,
                            )
                    for j in range(nj):
                        p_sb = ppool.tile([P, TO], DT, tag="po", name="p_sb")
                        nc.scalar.copy(out=p_sb[:], in_=psum_p[j][:])
                        nc.sync.dma_start(
                            out=pshard_d[off + j * P : off + (j + 1) * P, :],
                            in_=p_sb[:],
                        )

                # interleave: emit each dense 512-node tile right after the 4
                # spmm blocks that produce its agg1t columns
                st_widths = []
                off = 0
                while off < NPC:
                    w_ = min(512, NPC - off)
                    st_widths.append((off, w_))
                    off += w_
                b = 0
                for off, w_ in st_widths:
                    while b * P < off + w_:
                        spmm1_block(b)
                        b += 1
                    dense_tile(off, w_)

            # ---------------- AllGather P shards -> full P ----------------
            nc.gpsimd.collective_compute(
                "AllGather",
                mybir.AluOpType.bypass,
                replica_groups=[list(range(ncores))],
                ins=[pshard_d.opt()],
                outs=[pfull_d.opt()],
            )

            # ---------------- phase 2: SpMM2 over P -> out shard ----------
            with (
                tc.tile_pool(name="g2", bufs=2) as g2pool,
                tc.tile_pool(name="o", bufs=3) as opool,
                tc.tile_pool(name="po2", bufs=6, space="PSUM") as po_pool,
            ):
                ELEM = TO
                halves2 = (
                    (K_lo, pfull_d[:HALF, :], idx_lo_t, d_lo_t, w_lo_t, "glo", "lo"),
                    (K_hi, pfull_d[HALF:, :], idx_hi_t, d_hi_t, w_hi_t, "ghi", "hi"),
                )
                for b in range(BPC):
                    acc = po_pool.tile([P, TO], F32, space="PSUM", tag="acc")
                    nmm = K_lo + K_hi
                    i_mm = 0
                    for K, ptab, idx_t, d_t, w_t, gtag, hkey in halves2:
                        g = g2pool.tile([P, K * ELEM], DT, tag=gtag)
                        cm = cmin[hkey][b]
                        if cm < K:
                            nc.scalar.activation(
                                out=g[:, cm * ELEM : K * ELEM],
                                in_=zero_t[:, : (K - cm) * ELEM],
                                func=mybir.ActivationFunctionType.Relu,
                            )
                        nreg = creg[hkey]
                        nc.gpsimd.reg_load(nreg, cnt_tiles[hkey][0:1, b : b + 1])
                        nc.gpsimd.dma_gather(
                            g[:].rearrange("p (k e) -> p k e", e=ELEM),
                            ptab,
                            idx_t[:, b * K * 8 : (b + 1) * K * 8],
                            K * P,
                            nreg,
                            ELEM,
                            single_packet=False,
                        )
                        for k in range(K):
                            s_t = spool.tile([P, P], DT, tag="s")
                            nc.vector.tensor_scalar(
                                out=s_t[:],
                                in0=iota_t[:],
                                scalar1=d_t[:, b * K + k : b * K + k + 1],
                                scalar2=w_t[:, b * K + k : b * K + k + 1],
                                op0=mybir.AluOpType.is_equal,
                                op1=mybir.AluOpType.mult,
                            )
                            # out_block[dstlocal, f] += S.T @ g_chunk
                            nc.tensor.matmul(
                                out=acc[:],
                                lhsT=s_t[:],
                                rhs=g[:, ts(k, ELEM)],
                                start=(i_mm == 0),
                                stop=(i_mm == nmm - 1),
                            )
                            i_mm += 1
                    o_sb = opool.tile([P, TO], F32, tag="o")
                    nc.vector.tensor_copy(out=o_sb[:], in_=acc[:])
                    nc.sync.dma_start(out=oshard[ts(b, P), :], in_=o_sb[:])

    nc.compile()
    return nc


# --------------------------------------------------------------------------
# host orchestration
# --------------------------------------------------------------------------
def prep_inputs(x, edge_weight, W1, W2, mask1, mask2, src, dst):
    K_lo, K_hi, per_core, cmin = prep_graph(src, dst, edge_weight)
    x_lo, x_hi = _pad_table(np.asarray(x, np.float32), P)
    iota = _iota_tile()
    w1 = np.ascontiguousarray(np.asarray(W1, np.float32))
    w2 = np.ascontiguousarray(np.asarray(W2, np.float32))
    m1 = np.ascontiguousarray(
        np.asarray(mask1, np.float32).transpose(1, 0, 2).reshape(IN, T * HID)
    )
    m2 = np.ascontiguousarray(
        np.asarray(mask2, np.float32).transpose(1, 0, 2).reshape(HID, T * OUT)
    )

    in_maps = []
    for c in range(NCORES):
        m = dict(per_core[c])
        m.update(x_lo=x_lo, x_hi=x_hi, iota=iota, w1=w1, m1=m1, w2=w2, m2=m2)
        in_maps.append(m)
    return K_lo, K_hi, cmin, per_core, in_maps


def assemble_output(oshards):
    full = np.concatenate(oshards, axis=0)  # [NP_, 512]
    return np.ascontiguousarray(
        full[:N].reshape(N, T, OUT).transpose(1, 0, 2)
    ).astype(np.float32)


def kernel(x, edge_weight, W1, W2, mask1, mask2, src, dst):
    from concourse.bass_utils import run_bass_kernel_spmd

    K_lo, K_hi, cmin, per_core, in_maps = prep_inputs(
        x, edge_weight, W1, W2, mask1, mask2, src, dst
    )

    nc = build_kernel_fused(K_lo, K_hi, cmin)
    res = run_bass_kernel_spmd(nc, in_maps, core_ids=list(range(NCORES)))
    oshards = [res.results[c]["oshard"] for c in range(NCORES)]

    return assemble_output(oshards)


# revision 5
# speedup vs baseline: 1.9853x; 1.0418x over previous
"""Bass/Trainium2 kernel for nn_BayesianGNN (gnn_message_passing).

Computation (reference):
    agg1 = spmm(x, ew, src, dst)                       # [N, IN]
    for t in range(T):
        h_t   = relu(agg1 @ (W1 * mask1[t]))           # [N, HID]
        agg2  = spmm(h_t, ew, src, dst)                # [N, HID]
        out_t = agg2 @ (W2 * mask2[t])                 # [N, OUT]

Key algebraic restructure: spmm is linear, so
    out_t = spmm(relu(agg1 @ W1m_t) @ W2m_t)
i.e. the second spmm runs on OUT-wide (64) features instead of HID-wide
(128), and all T samples can be concatenated into one [N, T*OUT=512]
feature table P so the second spmm is done ONCE over 512-wide rows.

Sharding (8 cores): nodes are padded to 50176 = 392 blocks of 128 and
block-sharded: core c owns 49 consecutive blocks (6272 dst nodes) and all
edges pointing into them. ONE fused kernel launch per run (the axon/PJRT
dispatch has a large fixed latency, so launch count dominates wall time):

  phase 1: SpMM1 for its dst blocks via dma_gather of x rows + one-hot
    matmul segment-sum (S[e, j] = w_e * (dst_local_e == j), accumulated in
    PSUM per dst block, output transposed agg1T [feat, node]), then the
    dense per-sample MLP for its own nodes only, producing its shard of
    P [6272, 512] (node-major, samples in column groups of 64) into a
    DRAM bounce buffer.
  AllGather (on-device collective) of the P shards -> full P [50176, 512]
    in DRAM on every core.
  phase 2: SpMM2: dma_gather of P rows by src + the same one-hot matmul
    segment-sum -> out shard [6272, 512] (ExternalOutput).
  (host) de-interleave -> [T, N, OUT].

dma_gather indices are int16, so gather tables are split in two halves of
25088 rows; per (block, half) the edge list is padded to a fixed number of
128-edge chunks (K_lo/K_hi, computed from the data) so the program is
identical across all 8 cores (SPMD), with padding slots skipped via a
count register and the stale tail zero-filled up to the cross-core
minimum count.
"""

import sys

if "/opt/trn_rl_repo" not in sys.path:
    sys.path.insert(0, "/opt/trn_rl_repo")

import math

import numpy as np

import concourse.bass as bass
import concourse.tile as tile
from concourse import bacc, mybir
from concourse.bass import ts

F32 = mybir.dt.float32
BF16 = mybir.dt.bfloat16
I16 = mybir.dt.int16
I32 = mybir.dt.int32
USE_BF16 = True
DT = BF16 if USE_BF16 else F32

N, E = 50000, 800000
IN, HID, OUT, T = 96, 128, 64, 8
P = 128  # partitions
NCORES = 8
NBLK = 392  # node blocks of 128
NP_ = NBLK * P  # padded node count 50176
BPC = NBLK // NCORES  # blocks per core = 49
NPC = BPC * P  # nodes per core = 6272
HALF = NP_ // 2  # gather table half size = 25088
TO = T * OUT  # 512 = P feature width


def _np_dt():
    if USE_BF16:
        import ml_dtypes

        return np.dtype(ml_dtypes.bfloat16)
    return np.dtype(np.float32)


def _pad_table(a, cols):
    """[n, c] -> zero-padded [NP_, cols], split into (lo, hi) halves."""
    out = np.zeros((NP_, cols), _np_dt())
    out[: a.shape[0], : a.shape[1]] = a
    return out[:HALF], out[HALF:]


# --------------------------------------------------------------------------
# host-side graph prep
# --------------------------------------------------------------------------
def prep_graph(src, dst, ew):
    """Partition + pad edges into per-(core, block, half) chunk schedules.

    Returns (K_lo, K_hi, per_core, cmin) where per_core[c] holds the int16
    gather index arrays and the f32 dst-local / weight arrays in the
    layouts the kernel consumes.
    """
    src = np.asarray(src).astype(np.int64).ravel()
    dst = np.asarray(dst).astype(np.int64).ravel()
    ew = np.asarray(ew, dtype=np.float32).ravel()

    blk = dst >> 7
    half = (src >= HALF).astype(np.int64)
    order = np.lexsort((src, half, blk))
    sblk = blk[order]
    shalf = half[order]
    ssrc = src[order]
    sew = ew[order]
    sdl = (dst[order] & 127).astype(np.float32)

    cell = sblk * 2 + shalf
    counts = np.bincount(cell, minlength=NBLK * 2)
    K_lo = int(math.ceil(counts[0::2].max() / P))
    K_hi = int(math.ceil(counts[1::2].max() / P))

    cell_starts = np.zeros(NBLK * 2 + 1, np.int64)
    np.cumsum(counts, out=cell_starts[1:])
    pos = np.arange(E, dtype=np.int64) - cell_starts[cell]

    b_local = sblk % BPC
    core = sblk // BPC

    per_core = []
    for c in range(NCORES):
        d = {}
        for s, K, tag in ((0, K_lo, "lo"), (1, K_hi, "hi")):
            m = (core == c) & (shalf == s)
            nslot = BPC * K * P
            # pad with -1 (skipped by the gather)
            idxf = np.full(nslot, -1, np.int16)
            df = np.zeros(nslot, np.float32)
            wf = np.zeros(nslot, np.float32)
            slots = b_local[m] * (K * P) + pos[m]
            idxf[slots] = (ssrc[m] - s * HALF).astype(np.int16)
            df[slots] = sdl[m]
            wf[slots] = sew[m]
            cnt = np.bincount(b_local[m], minlength=BPC).astype(np.int32)
            # never let a gather have zero valid indices
            empty = cnt == 0
            if empty.any():
                for b in np.nonzero(empty)[0]:
                    idxf[b * K * P] = 0
                cnt[empty] = 1
            d["idx_" + tag] = np.ascontiguousarray(
                np.tile(idxf.reshape(BPC * K * 8, 16).T, (8, 1))
            )
            d["cnt_" + tag] = np.ascontiguousarray(cnt.reshape(1, BPC))
            d["d_" + tag] = np.ascontiguousarray(
                df.reshape(BPC, K, P).transpose(2, 0, 1).reshape(P, BPC * K)
            )
            d["w_" + tag] = np.ascontiguousarray(
                wf.reshape(BPC, K, P).transpose(2, 0, 1).reshape(P, BPC * K)
            )
        per_core.append(d)
    cmin = {}
    for tag in ("lo", "hi"):
        allcnt = np.stack([pc["cnt_" + tag][0] for pc in per_core])  # [NCORES, BPC]
        cmin[tag] = (allcnt.min(axis=0) // P).astype(int).tolist()
    return K_lo, K_hi, per_core, cmin


def _iota_tile():
    return np.tile(np.arange(P, dtype=np.float32), (P, 1))


# --------------------------------------------------------------------------
# fused kernel: SpMM1 + MLP -> P shard; AllGather; SpMM2 -> out shard
# --------------------------------------------------------------------------
def build_kernel_fused(K_lo, K_hi, cmin, ncores=NCORES):
    nc = bacc.Bacc("TRN2", target_bir_lowering=False, debug=False)

    x_lo = nc.dram_tensor("x_lo", [HALF, P], DT, kind="ExternalInput")
    x_hi = nc.dram_tensor("x_hi", [HALF, P], DT, kind="ExternalInput")
    idx_lo = nc.dram_tensor("idx_lo", [P, BPC * K_lo * 8], I16, kind="ExternalInput")
    idx_hi = nc.dram_tensor("idx_hi", [P, BPC * K_hi * 8], I16, kind="ExternalInput")
    d_lo = nc.dram_tensor("d_lo", [P, BPC * K_lo], F32, kind="ExternalInput")
    w_lo = nc.dram_tensor("w_lo", [P, BPC * K_lo], F32, kind="ExternalInput")
    d_hi = nc.dram_tensor("d_hi", [P, BPC * K_hi], F32, kind="ExternalInput")
    w_hi = nc.dram_tensor("w_hi", [P, BPC * K_hi], F32, kind="ExternalInput")
    cnt_lo = nc.dram_tensor("cnt_lo", [1, BPC], I32, kind="ExternalInput")
    cnt_hi = nc.dram_tensor("cnt_hi", [1, BPC], I32, kind="ExternalInput")
    iota = nc.dram_tensor("iota", [P, P], F32, kind="ExternalInput")
    w1 = nc.dram_tensor("w1", [IN, HID], F32, kind="ExternalInput")
    m1 = nc.dram_tensor("m1", [IN, T * HID], F32, kind="ExternalInput")
    w2 = nc.dram_tensor("w2", [HID, OUT], F32, kind="ExternalInput")
    m2 = nc.dram_tensor("m2", [HID, T * OUT], F32, kind="ExternalInput")
    oshard = nc.dram_tensor("oshard", [NPC, TO], F32, kind="ExternalOutput")

    with tile.TileContext(nc) as tc:
        with (
            tc.tile_pool(name="const", bufs=1) as cpool,
            tc.tile_pool(name="dram", bufs=1, space="DRAM") as dpool,
            tc.tile_pool(name="s", bufs=4) as spool,
        ):
            pshard_d = dpool.tile([NPC, TO], DT, name="pshard_d")
            pfull_d = dpool.tile([NP_, TO], DT, name="pfull_d", addr_space="Shared")

            # ---- load constants
            def load(t_dram, shape, dtype=F32):
                nm = f"c_{t_dram.name}"
                t_sb = cpool.tile([P, shape[1]], dtype, name=nm, tag=nm)
                nc.sync.dma_start(out=t_sb[: shape[0], :], in_=t_dram[:])
                return t_sb

            idx_lo_t = load(idx_lo, [P, BPC * K_lo * 8], I16)
            idx_hi_t = load(idx_hi, [P, BPC * K_hi * 8], I16)
            d_lo_t = load(d_lo, [P, BPC * K_lo])
            w_lo_t = load(w_lo, [P, BPC * K_lo])
            d_hi_t = load(d_hi, [P, BPC * K_hi])
            w_hi_t = load(w_hi, [P, BPC * K_hi])
            cnt_lo_t = load(cnt_lo, [1, BPC], I32)
            cnt_hi_t = load(cnt_hi, [1, BPC], I32)
            iota_t = load(iota, [P, P])
            ztail = max(
                [(K_lo - c) for c in cmin["lo"]] + [(K_hi - c) for c in cmin["hi"]]
            )
            # zero tail sized for the widest use (phase 2, ELEM=TO)
            zero_t = cpool.tile([P, ztail * TO], DT, name="zero_t", tag="zero_t")
            nc.vector.memset(zero_t[:], 0.0)
            creg = {
                "lo": nc.gpsimd.alloc_register("cnt_reg_lo"),
                "hi": nc.gpsimd.alloc_register("cnt_reg_hi"),
            }
            cnt_tiles = {"lo": cnt_lo_t, "hi": cnt_hi_t}
            w1_t = load(w1, [IN, HID])
            m1_t = load(m1, [IN, T * HID])
            w2_t = load(w2, [HID, OUT])
            m2_t = load(m2, [HID, T * OUT])

            # masked weights; rows IN..P of w1m stay zero
            w1m = cpool.tile([P, T * HID], DT)
            nc.gpsimd.memset(w1m[:], 0.0)
            for t in range(T):
                nc.vector.tensor_tensor(
                    out=w1m[:IN, ts(t, HID)],
                    in0=w1_t[:IN, :],
                    in1=m1_t[:IN, ts(t, HID)],
                    op=mybir.AluOpType.mult,
                )
            w2m = cpool.tile([P, T * OUT], DT)
            for t in range(T):
                nc.vector.tensor_tensor(
                    out=w2m[:, ts(t, OUT)],
                    in0=w2_t[:, :],
                    in1=m2_t[:, ts(t, OUT)],
                    op=mybir.AluOpType.mult,
                )

            # agg1 transposed [feat, node] for the whole shard, kept in SBUF
            agg1t = cpool.tile([P, NPC], DT)

            # ---------------- phase 1: SpMM1 + dense MLP -> pshard_d ----
            with (
                tc.tile_pool(name="glo1", bufs=3) as gpool,
                tc.tile_pool(name="h", bufs=3) as hpool,
                tc.tile_pool(name="po", bufs=2) as ppool,
                tc.tile_pool(name="acc", bufs=2, space="PSUM") as acc_pool,
                tc.tile_pool(name="ph", bufs=2, space="PSUM") as ph_pool,
                tc.tile_pool(name="pp", bufs=1, space="PSUM") as pp_pool,
            ):
                halves1 = (
                    (K_lo, x_lo, idx_lo_t, d_lo_t, w_lo_t, "glo", "lo"),
                    (K_hi, x_hi, idx_hi_t, d_hi_t, w_hi_t, "ghi", "hi"),
                )

                def spmm1_block(b):
                    ELEM = P
                    acc = acc_pool.tile([P, P], F32, space="PSUM", tag="acc", name="acc")
                    nmm = K_lo + K_hi
                    i_mm = 0
                    for K, xtab, idx_t, d_t, w_t, gtag, hkey in halves1:
                        g = gpool.tile([P, K * ELEM], DT, tag=gtag, name=gtag)
                        cm = cmin[hkey][b]
                        if cm < K:
                            nc.scalar.activation(
                                out=g[:, cm * ELEM : K * ELEM],
                                in_=zero_t[:, : (K - cm) * ELEM],
                                func=mybir.ActivationFunctionType.Relu,
                            )
                        nreg = creg[hkey]
                        nc.gpsimd.reg_load(nreg, cnt_tiles[hkey][0:1, b : b + 1])
                        nc.gpsimd.dma_gather(
                            g[:].rearrange("p (k e) -> p k e", e=ELEM),
                            xtab[:],
                            idx_t[:, b * K * 8 : (b + 1) * K * 8],
                            K * P,
                            nreg,
                            ELEM,
                            single_packet=False,
                        )
                        for k in range(K):
                            s_t = spool.tile([P, P], DT, tag="s", name="s_t")
                            nc.vector.tensor_scalar(
                                out=s_t[:],
                                in0=iota_t[:],
                                scalar1=d_t[:, b * K + k : b * K + k + 1],
                                scalar2=w_t[:, b * K + k : b * K + k + 1],
                                op0=mybir.AluOpType.is_equal,
                                op1=mybir.AluOpType.mult,
                            )
                            # agg1t_block[feat, dstlocal] += g_chunk.T @ S
                            nc.tensor.matmul(
                                out=acc[:],
                                lhsT=g[:, ts(k, ELEM)],
                                rhs=s_t[:],
                                start=(i_mm == 0),
                                stop=(i_mm == nmm - 1),
                            )
                            i_mm += 1
                    nc.vector.tensor_copy(out=agg1t[:, ts(b, P)], in_=acc[:])

                def dense_tile(off, w_):
                    nj = w_ // P
                    psum_p = [
                        pp_pool.tile(
                            [P, TO], F32, space="PSUM", tag=f"pp{j}", name=f"pp{j}"
                        )
                        for j in range(nj)
                    ]
                    for t in range(T):
                        psum_h = ph_pool.tile(
                            [P, w_], F32, space="PSUM", tag="ph", name="ph"
                        )
                        nc.tensor.matmul(
                            out=psum_h[:],
                            lhsT=w1m[:, ts(t, HID)],
                            rhs=agg1t[:, off : off + w_],
                            start=True,
                            stop=True,
                        )
                        h_sb = hpool.tile([P, w_], DT, tag="h", name="h_sb")
                        nc.scalar.activation(
                            out=h_sb[:],
                            in_=psum_h[:],
                            func=mybir.ActivationFunctionType.Relu,
                        )
                        for j in range(nj):
                            nc.tensor.matmul(
                                out=psum_p[j][:, ts(t, OUT)],
                                lhsT=h_sb[:, ts(j, P)],
                                rhs=w2m[:, ts(t, OUT)],
                                start=True,
                                stop=True,
                            )
                    for j in range(nj):
                        p_sb = ppool.tile([P, TO], DT, tag="po", name="p_sb")
                        nc.scalar.copy(out=p_sb[:], in_=psum_p[j][:])
                        nc.sync.dma_start(
                            out=pshard_d[off + j * P : off + (j + 1) * P, :],
                            in_=p_sb[:],
                        )

                # interleave: emit each dense 512-node tile right after the 4
                # spmm blocks that produce its agg1t columns
                st_widths = []
                off = 0
                while off < NPC:
                    w_ = min(512, NPC - off)
                    st_widths.append((off, w_))
                    off += w_
                b = 0
                for off, w_ in st_widths:
                    while b * P < off + w_:
                        spmm1_block(b)
                        b += 1
                    dense_tile(off, w_)

            # ---------------- AllGather P shards -> full P ----------------
            nc.gpsimd.collective_compute(
                "AllGather",
                mybir.AluOpType.bypass,
                replica_groups=[list(range(ncores))],
                ins=[pshard_d.opt()],
                outs=[pfull_d.opt()],
            )

            # ---------------- phase 2: SpMM2 over P -> out shard ----------
            with (
                tc.tile_pool(name="g2", bufs=3) as g2pool,
                tc.tile_pool(name="o", bufs=3) as opool,
                tc.tile_pool(name="po2", bufs=6, space="PSUM") as po_pool,
            ):
                ELEM = TO
                halves2 = (
                    (K_lo, pfull_d[:HALF, :], idx_lo_t, d_lo_t, w_lo_t, "glo", "lo"),
                    (K_hi, pfull_d[HALF:, :], idx_hi_t, d_hi_t, w_hi_t, "ghi", "hi"),
                )
                for b in range(BPC):
                    acc = po_pool.tile([P, TO], F32, space="PSUM", tag="acc")
                    nmm = K_lo + K_hi
                    i_mm = 0
                    for K, ptab, idx_t, d_t, w_t, gtag, hkey in halves2:
                        g = g2pool.tile([P, K * ELEM], DT, tag=gtag)
                        cm = cmin[hkey][b]
                        if cm < K:
                            nc.scalar.activation(
                                out=g[:, cm * ELEM : K * ELEM],
                                in_=zero_t[:, : (K - cm) * ELEM],
                                func=mybir.ActivationFunctionType.Relu,
                            )
                        nreg = creg[hkey]
                        nc.gpsimd.reg_load(nreg, cnt_tiles[hkey][0:1, b : b + 1])
                        nc.gpsimd.dma_gather(
                            g[:].rearrange("p (k e) -> p k e", e=ELEM),
                            ptab,
                            idx_t[:, b * K * 8 : (b + 1) * K * 8],
                            K * P,
                            nreg,
                            ELEM,
                            single_packet=False,
                        )
                        for k in range(K):
                            s_t = spool.tile([P, P], DT, tag="s")
                            nc.vector.tensor_scalar(
                                out=s_t[:],
                                in0=iota_t[:],
                                scalar1=d_t[:, b * K + k : b * K + k + 1],
                                scalar2=w_t[:, b * K + k : b * K + k + 1],
                                op0=mybir.AluOpType.is_equal,
                                op1=mybir.AluOpType.mult,
                            )
                            # out_block[dstlocal, f] += S.T @ g_chunk
                            nc.tensor.matmul(
                                out=acc[:],
                                lhsT=s_t[:],
                                rhs=g[:, ts(k, ELEM)],
                                start=(i_mm == 0),
                                stop=(i_mm == nmm - 1),
                            )
                            i_mm += 1
                    o_sb = opool.tile([P, TO], F32, tag="o")
                    nc.vector.tensor_copy(out=o_sb[:], in_=acc[:])
                    nc.sync.dma_start(out=oshard[ts(b, P), :], in_=o_sb[:])

    nc.compile()
    return nc


# --------------------------------------------------------------------------
# host orchestration
# --------------------------------------------------------------------------
def prep_inputs(x, edge_weight, W1, W2, mask1, mask2, src, dst):
    K_lo, K_hi, per_core, cmin = prep_graph(src, dst, edge_weight)
    x_lo, x_hi = _pad_table(np.asarray(x, np.float32), P)
    iota = _iota_tile()
    w1 = np.ascontiguousarray(np.asarray(W1, np.float32))
    w2 = np.ascontiguousarray(np.asarray(W2, np.float32))
    m1 = np.ascontiguousarray(
        np.asarray(mask1, np.float32).transpose(1, 0, 2).reshape(IN, T * HID)
    )
    m2 = np.ascontiguousarray(
        np.asarray(mask2, np.float32).transpose(1, 0, 2).reshape(HID, T * OUT)
    )

    in_maps = []
    for c in range(NCORES):
        m = dict(per_core[c])
        m.update(x_lo=x_lo, x_hi=x_hi, iota=iota, w1=w1, m1=m1, w2=w2, m2=m2)
        in_maps.append(m)
    return K_lo, K_hi, cmin, per_core, in_maps


def assemble_output(oshards):
    full = np.concatenate(oshards, axis=0)  # [NP_, 512]
    return np.ascontiguousarray(
        full[:N].reshape(N, T, OUT).transpose(1, 0, 2)
    ).astype(np.float32)


def _kernel_once(x, edge_weight, W1, W2, mask1, mask2, src, dst):
    from concourse.bass_utils import run_bass_kernel_spmd

    K_lo, K_hi, cmin, per_core, in_maps = prep_inputs(
        x, edge_weight, W1, W2, mask1, mask2, src, dst
    )

    nc = build_kernel_fused(K_lo, K_hi, cmin)
    res = run_bass_kernel_spmd(nc, in_maps, core_ids=list(range(NCORES)))
    oshards = [res.results[c]["oshard"] for c in range(NCORES)]

    return assemble_output(oshards)


_CHILD_SRC = r"""
import sys
sys.path.insert(0, "/opt/trn_rl_repo")
import importlib.util
import numpy as np

kpath, inpath, outpath = sys.argv[1], sys.argv[2], sys.argv[3]
spec = importlib.util.spec_from_file_location("_kernel_child_mod", kpath)
mod = importlib.util.module_from_spec(spec)
spec.loader.exec_module(mod)
d = np.load(inpath)
out = mod._kernel_once(**{k: d[k] for k in d.files})
np.save(outpath, out)
"""


def kernel(x, edge_weight, W1, W2, mask1, mask2, src, dst):
    """Full-input entry point.

    The first collective execution in a fresh process occasionally dies with
    a transient NRT mesh-desync; a retry in a *new* process reliably
    recovers. Attempt in-process first, then fall back to subprocess
    retries with the same inputs.
    """
    inputs = dict(
        x=x, edge_weight=edge_weight, W1=W1, W2=W2,
        mask1=mask1, mask2=mask2, src=src, dst=dst,
    )
    try:
        return _kernel_once(**inputs)
    except Exception as e:  # noqa: BLE001 - any device failure -> retry
        print(f"kernel: in-process attempt failed ({type(e).__name__}); "
              f"retrying in subprocess", file=sys.stderr)

    import os
    import subprocess
    import tempfile

    tmp = tempfile.mkdtemp(prefix="bgnn_kernel_")
    inpath = os.path.join(tmp, "in.npz")
    np.savez(inpath, **{k: np.asarray(v) for k, v in inputs.items()})
    childpath = os.path.join(tmp, "child.py")
    with open(childpath, "w") as f:
        f.write(_CHILD_SRC)
    kpath = os.path.abspath(__file__)

    last = None
    for attempt in range(3):
        outpath = os.path.join(tmp, f"out{attempt}.npy")
        try:
            p = subprocess.run(
                [sys.executable, childpath, kpath, inpath, outpath],
                capture_output=True, text=True, timeout=2400,
            )
            if p.returncode == 0 and os.path.exists(outpath):
                return np.load(outpath)
            last = RuntimeError(
                f"child attempt {attempt} rc={p.returncode}: "
                f"{(p.stderr or '')[-400:]}"
            )
        except Exception as e:  # noqa: BLE001
            last = e
        print(f"kernel: subprocess attempt {attempt} failed", file=sys.stderr)
    raise last
